# revision 7
# baseline (speedup 1.0000x reference)
"""BiLSTM-CRF NLL kernel for 8 Trainium2 NeuronCores.

Contract: kernel(**inputs) takes the FULL unsharded inputs (as produced by the
reference setup_inputs()) and returns the FULL output (a float32 scalar).

Sharding strategy (hardcoded): data-parallel over the batch dim. B=64 is split
into 8 shards of 8 sequences; LSTM/CRF parameters are replicated on every core.
Each core computes the total NLL of its 8 sequences on-device; the host sums
the 8 partial scalars (the "unshard" step).

Per-core pipeline (all on device):
  0. embedding gather via indirect DMA (token-major [128, E] tiles),
     PE transposes to xT [E, tokens]
  1. input projections g_ih = W_ih @ x + b for all tokens as dense matmuls
     (f32), stored fp16 in SBUF, gate chunks pre-permuted to (i,i,f,f,o,o,g,g)
  2. the two LSTM recurrences (fwd / bwd), interleaved. Per step:
     16 bf16 [128,128] weight tiles x [128,8] h -> PSUM [128,64], plus one
     identity-matmul that accumulates the precomputed g_ih into the same PSUM
     bank; sigmoid/tanh on ACT; cell update split between DVE and GPSIMD.
     h states are written bf16 straight into the h history buffer that serves
     as both next-step matmul operand and emission-matmul operand.
  3. emissions transposed [9, tokens] = W_tag.T-chunks @ h, E = exp(emis - mu)
  4. CRF in exp space: the forward-algorithm logsumexp becomes
     A' = (exp(trans).T @ A) * E_t  -- a [9,9]x[9,8] matmul plus one
     elementwise multiply per step. Meet-in-the-middle: a forward chain
     (t=0..127) and a backward chain (t=255..128) run concurrently, halving
     the sequential depth; logZ = log(sum_i A_127 * B_127) + 256*mu.
     The constant shift mu=log(9) keeps exp-space values in f32 range and
     cancels exactly in logZ.
  5. gold path score via one-hot tensors (host-encoded from tags) and
     matmuls/reductions; output = sum_b (logZ_b - score_b) as [1,1] f32.
"""

import functools
import math
import os
import sys

import numpy as np

for _p in ("/opt/trn_rl_repo", "/opt/pypackages"):
    if _p not in sys.path and os.path.isdir(_p):
        sys.path.append(_p)

import ml_dtypes  # noqa: E402

import concourse.bass as bass  # noqa: E402
import concourse.mybir as mybir  # noqa: E402
import concourse.tile as tile  # noqa: E402
from concourse import bacc  # noqa: E402
from concourse.bass import IndirectOffsetOnAxis  # noqa: E402
from concourse.bass_utils import run_bass_kernel_spmd  # noqa: E402

F32 = mybir.dt.float32
F16 = mybir.dt.float16
BF16 = mybir.dt.bfloat16
I32 = mybir.dt.int32
AF = mybir.ActivationFunctionType
OP = mybir.AluOpType

# Problem constants (hardcoded per the task contract).
B, S, V, E, H, T = 64, 256, 50000, 256, 512, 9
HD = H // 2               # 256 per-direction hidden
NCORES = 8
BL = B // NCORES          # 8 sequences per core
TOK = BL * S              # 2048 tokens per core
NCH = TOK // 128          # 16 gather chunks of 128 tokens
MU = math.log(9.0)        # exp-space drift compensation, cancels exactly
# gate chunk permutation: original (i0 i1 f0 f1 g0 g1 o0 o1) -> (i i f f o o g g)
PERM = [0, 1, 2, 3, 6, 7, 4, 5]
HSLOT = 16                # one h slot = 2 hd-chunks x 8 batch
NSTEP_CH = S // 64        # 4 phase-1 n-chunks of 512 tokens


def _emit_step(nc, d, t, whh, gih, hall, idf16, c_state, work, ps_pool):
    """Emit one LSTM step for direction d ('f' or 'b') processing token t."""
    rd = t if d == "f" else t + 1
    wr = t + 1 if d == "f" else t
    ps = ps_pool[d].tile([128, 64], F32, tag=f"st{d}", name=f"ps{d}")
    for m in range(8):
        for k in range(2):
            nc.tensor.matmul(
                out=ps[:, m * 8:(m + 1) * 8],
                lhsT=whh[d][k][:, m * 128:(m + 1) * 128],
                rhs=hall[d][:, rd * HSLOT + k * 8: rd * HSLOT + k * 8 + 8],
                start=(k == 0),
                stop=False,
                skip_group_check=True,
            )
    # accumulate g_ih (+bias, already folded) via identity matmul
    nc.tensor.matmul(
        out=ps[:, :],
        lhsT=idf16[:],
        rhs=gih[d][:, t * 64:(t + 1) * 64],
        start=False,
        stop=True,
        skip_group_check=True,
    )
    sig = work.tile([128, 48], F32, tag=f"sig{d}")
    nc.scalar.activation(sig[:], ps[:, 0:48], AF.Sigmoid)
    tg = work.tile([128, 16], F32, tag=f"tg{d}")
    nc.scalar.activation(tg[:], ps[:, 48:64], AF.Tanh)
    u = work.tile([128, 16], F32, tag=f"u{d}")
    nc.gpsimd.tensor_tensor(u[:], sig[:, 0:16], tg[:], op=OP.mult)
    v = work.tile([128, 16], F32, tag=f"v{d}")
    nc.vector.tensor_tensor(v[:], sig[:, 16:32], c_state[d][:], op=OP.mult)
    nc.vector.tensor_tensor(c_state[d][:], u[:], v[:], op=OP.add)
    tcn = work.tile([128, 16], F32, tag=f"tc{d}")
    nc.scalar.activation(tcn[:], c_state[d][:], AF.Tanh)
    nc.gpsimd.tensor_tensor(
        hall[d][:, wr * HSLOT:(wr + 1) * HSLOT], sig[:, 32:48], tcn[:], op=OP.mult
    )


@functools.lru_cache(maxsize=2)
def _build(seq_len=S):
    """Build the Bass program (same SPMD program for all 8 cores)."""
    global S, TOK, NCH, NSTEP_CH
    assert seq_len == S, "builder is specialized to S=256"

    nc = bacc.Bacc("TRN2", target_bir_lowering=False, debug=False)

    # ---- DRAM I/O ----
    emb_d = nc.dram_tensor("emb", [V, E], F32, kind="ExternalInput")
    idx_d = nc.dram_tensor("idx", [128, NCH], I32, kind="ExternalInput")
    wih_d = {d: nc.dram_tensor(f"wih_{d}", [E, 4 * HD], F32, kind="ExternalInput")
             for d in "fb"}
    whh_d = {d: nc.dram_tensor(f"whh_{d}", [HD, 4 * HD], BF16, kind="ExternalInput")
             for d in "fb"}
    br_d = {d: nc.dram_tensor(f"br_{d}", [128, 8], F32, kind="ExternalInput")
            for d in "fb"}
    wtag_d = nc.dram_tensor("wtagT", [H, T], BF16, kind="ExternalInput")
    btag_d = nc.dram_tensor("btag", [T, 1], F32, kind="ExternalInput")
    start_d = nc.dram_tensor("startv", [T, 1], F32, kind="ExternalInput")
    end_d = nc.dram_tensor("endv", [T, 1], F32, kind="ExternalInput")
    trans_d = nc.dram_tensor("transm", [T, T], F32, kind="ExternalInput")
    transT_d = nc.dram_tensor("transmT", [T, T], F32, kind="ExternalInput")
    ohc_d = nc.dram_tensor("ohc", [T, TOK], F32, kind="ExternalInput")
    ohn_d = nc.dram_tensor("ohn", [T, TOK], F32, kind="ExternalInput")
    idf32_d = nc.dram_tensor("idf32", [128, 128], F32, kind="ExternalInput")
    idf16_d = nc.dram_tensor("idf16", [128, 128], F16, kind="ExternalInput")
    out_d = nc.dram_tensor("out", [1, 1], F32, kind="ExternalOutput")

    with tile.TileContext(nc) as tc:
        with (
            tc.tile_pool(name="pers", bufs=1) as pers,
            tc.tile_pool(name="work", bufs=3) as work,
            tc.tile_pool(name="psbig", bufs=2, space="PSUM") as ps_big,
            tc.tile_pool(name="pstp", bufs=2, space="PSUM") as ps_tp,
            tc.tile_pool(name="psf", bufs=2, space="PSUM") as ps_f,
            tc.tile_pool(name="psb", bufs=2, space="PSUM") as ps_b,
        ):
            ps_pool = {"f": ps_f, "b": ps_b}

            # ---- persistent SBUF ----
            idx_sb = pers.tile([128, NCH], I32, tag="idx")
            nc.sync.dma_start(idx_sb[:], idx_d[:])
            idf32 = pers.tile([128, 128], F32, tag="idf32")
            nc.sync.dma_start(idf32[:], idf32_d[:])
            idf16 = pers.tile([128, 128], F16, tag="idf16")
            nc.sync.dma_start(idf16[:], idf16_d[:])

            wih, whh, br, gih, hall, c_state = {}, {}, {}, {}, {}, {}
            for d in "fb":
                wih[d] = [pers.tile([128, 4 * HD], F32, tag=f"wih{d}{k}",
                                    name=f"wih{d}{k}") for k in range(2)]
                for k in range(2):
                    nc.sync.dma_start(wih[d][k][:], wih_d[d][k * 128:(k + 1) * 128, :])
                whh[d] = [pers.tile([128, 4 * HD], BF16, tag=f"whh{d}{k}",
                                    name=f"whh{d}{k}") for k in range(2)]
                for k in range(2):
                    nc.sync.dma_start(whh[d][k][:], whh_d[d][k * 128:(k + 1) * 128, :])
                br[d] = pers.tile([128, 8], F32, tag=f"br{d}", name=f"br{d}")
                nc.sync.dma_start(br[d][:], br_d[d][:])
                gih[d] = pers.tile([128, S * 64], F16, tag=f"gih{d}", name=f"gih{d}")
                hall[d] = pers.tile([128, (S + 1) * HSLOT], BF16, tag=f"hall{d}", name=f"hall{d}")
                c_state[d] = pers.tile([128, 16], F32, tag=f"c{d}", name=f"c{d}")
                nc.vector.memset(c_state[d][:], 0.0)
            # zero initial h slots (fwd reads slot 0, bwd reads slot S)
            nc.vector.memset(hall["f"][:, 0:HSLOT], 0.0)
            nc.vector.memset(hall["b"][:, S * HSLOT:(S + 1) * HSLOT], 0.0)

            wtagT = [pers.tile([128, T], BF16, tag=f"wtag{kk}", name=f"wtag{kk}")
                      for kk in range(4)]
            for kk in range(4):
                nc.sync.dma_start(wtagT[kk][:], wtag_d[kk * 128:(kk + 1) * 128, :])
            btag = pers.tile([T, 1], F32, tag="btag")
            nc.sync.dma_start(btag[:], btag_d[:])
            startv = pers.tile([T, 1], F32, tag="startv")
            nc.sync.dma_start(startv[:], start_d[:])
            endv = pers.tile([T, 1], F32, tag="endv")
            nc.sync.dma_start(endv[:], end_d[:])
            transm = pers.tile([T, T], F32, tag="transm")
            nc.sync.dma_start(transm[:], trans_d[:])
            transmT = pers.tile([T, T], F32, tag="transmT")
            nc.sync.dma_start(transmT[:], transT_d[:])
            ohc = pers.tile([T, TOK], F32, tag="ohc")
            nc.sync.dma_start(ohc[:], ohc_d[:])
            ohn = pers.tile([T, TOK], F32, tag="ohn")
            nc.sync.dma_start(ohn[:], ohn_d[:])
            ones9 = pers.tile([T, 1], F32, tag="ones9")
            nc.vector.memset(ones9[:], 1.0)
            ones98 = pers.tile([T, 8], F32, tag="ones98")
            nc.vector.memset(ones98[:], 1.0)

            # ---- phase 0: embedding gather + transpose to xT [E, TOK] ----
            xg = pers.tile([128, NCH * E], F32, tag="xg")
            xT = [pers.tile([128, TOK], F32, tag=f"xT{k}", name=f"xT{k}")
                  for k in range(2)]
            for ch in range(NCH):
                nc.gpsimd.indirect_dma_start(
                    out=xg[:, ch * E:(ch + 1) * E],
                    out_offset=None,
                    in_=emb_d[:],
                    in_offset=IndirectOffsetOnAxis(ap=idx_sb[:, ch:ch + 1], axis=0),
                )
            for ch in range(NCH):
                for k in range(2):
                    pst = ps_tp.tile([128, 128], F32, tag="tp")
                    nc.tensor.transpose(
                        out=pst[:],
                        in_=xg[:, ch * E + k * 128: ch * E + (k + 1) * 128],
                        identity=idf32[:],
                    )
                    nc.vector.tensor_copy(xT[k][:, ch * 128:(ch + 1) * 128], pst[:])

            # ---- phase 1: g_ih = W_ih @ x + b, fp16, layout (t, m, b) ----
            for d in "fb":
                n_order = [0, 3, 1, 2] if d == "f" else [3, 0, 2, 1]
                for n in n_order:
                    for m in range(8):
                        psg = ps_big.tile([128, 512], F32, tag="big")
                        for k in range(2):
                            nc.tensor.matmul(
                                out=psg[:],
                                lhsT=wih[d][k][:, m * 128:(m + 1) * 128],
                                rhs=xT[k][:, n * 512:(n + 1) * 512],
                                start=(k == 0),
                                stop=(k == 1),
                            )
                        dst = gih[d][:].rearrange(
                            "p (t m b) -> p t m b", t=S, m=8, b=8
                        )[:, n * 64:(n + 1) * 64, m, :]
                        src = psg[:].rearrange("p (t b) -> p t b", t=64, b=8)
                        if m % 2 == 0:
                            nc.vector.tensor_scalar_add(dst, src, br[d][:, m:m + 1])
                        else:
                            nc.scalar.activation(dst, src, AF.Identity,
                                                 bias=br[d][:, m:m + 1])

            # ---- phase 2: the two LSTM recurrences, interleaved ----
            for t in range(S):
                _emit_step(nc, "f", t, whh, gih, hall, idf16, c_state, work, ps_pool)
                _emit_step(nc, "b", S - 1 - t, whh, gih, hall, idf16, c_state,
                           work, ps_pool)

            # ---- phase 3: emissions (transposed) + E = exp(emis - mu) ----
            emisraw = pers.tile([T, TOK], F32, tag="emisraw")
            ebuf = pers.tile([T, TOK], F32, tag="ebuf")
            hview = {d: hall[d][:].rearrange("p (s c b) -> p s c b", s=S + 1, c=2, b=8)
                     for d in "fb"}
            for n in range(4):
                pse = ps_big.tile([T, 512], F32, tag="big")
                for kk in range(4):
                    d = "f" if kk < 2 else "b"
                    c = kk % 2
                    lo = n * 64 + (1 if d == "f" else 0)
                    rhs = hview[d][:, lo:lo + 64, c, :]
                    nc.tensor.matmul(
                        out=pse[:],
                        lhsT=wtagT[kk][:],
                        rhs=rhs,
                        start=(kk == 0),
                        stop=(kk == 3),
                    )
                nc.vector.tensor_scalar_add(
                    emisraw[:, n * 512:(n + 1) * 512], pse[:], btag[:, 0:1]
                )
            negmu = pers.tile([T, 1], F32, tag="negmu")
            nc.vector.memset(negmu[:], -MU)
            nc.scalar.activation(ebuf[:], emisraw[:], AF.Exp, bias=negmu[:, 0:1])

            # ---- phase 4: gold path score ----
            tmp9 = pers.tile([T, TOK], F32, tag="tmp9")
            nc.vector.tensor_tensor(tmp9[:], emisraw[:], ohc[:], op=OP.mult)
            gm = pers.tile([T, 8], F32, tag="gm")
            nc.vector.tensor_reduce(
                gm[:],
                tmp9[:].rearrange("p (t b) -> p b t", t=S, b=8),
                axis=mybir.AxisListType.X,
                op=OP.add,
            )
            for n in range(4):
                psg2 = ps_big.tile([T, 512], F32, tag="big")
                nc.tensor.matmul(
                    out=psg2[:],
                    lhsT=transm[:],
                    rhs=ohc[:, n * 512:(n + 1) * 512],
                    start=True,
                    stop=True,
                )
                nc.vector.tensor_tensor(
                    tmp9[:, n * 512:(n + 1) * 512], psg2[:],
                    ohn[:, n * 512:(n + 1) * 512], op=OP.mult,
                )
            gtr = pers.tile([T, 8], F32, tag="gtr")
            nc.vector.tensor_reduce(
                gtr[:],
                tmp9[:].rearrange("p (t b) -> p b t", t=S, b=8),
                axis=mybir.AxisListType.X,
                op=OP.add,
            )
            gse = pers.tile([T, 8], F32, tag="gse")
            nc.vector.tensor_scalar(
                gse[:], ohc[:, 0:8], scalar1=startv[:, 0:1], scalar2=None,
                op0=OP.mult,
            )
            gee = pers.tile([T, 8], F32, tag="gee")
            nc.vector.tensor_scalar(
                gee[:], ohc[:, (S - 1) * 8:S * 8], scalar1=endv[:, 0:1],
                scalar2=None, op0=OP.mult,
            )
            nc.vector.tensor_tensor(gm[:], gm[:], gtr[:], op=OP.add)
            nc.vector.tensor_tensor(gse[:], gse[:], gee[:], op=OP.add)
            nc.vector.tensor_tensor(gm[:], gm[:], gse[:], op=OP.add)
            ps_sc = ps_tp.tile([1, 8], F32, tag="tp")
            nc.tensor.matmul(out=ps_sc[:], lhsT=ones9[:], rhs=gm[:],
                             start=True, stop=True)
            score_sb = pers.tile([1, 8], F32, tag="score")
            nc.vector.tensor_copy(score_sb[:], ps_sc[:])

            # ---- phase 5: CRF forward/backward exp-space chains ----
            expT = pers.tile([T, T], F32, tag="expT")
            nc.scalar.activation(expT[:], transm[:], AF.Exp)
            expTT = pers.tile([T, T], F32, tag="expTT")
            nc.scalar.activation(expTT[:], transmT[:], AF.Exp)
            exps = pers.tile([T, 1], F32, tag="exps")
            nc.scalar.activation(exps[:], startv[:], AF.Exp)
            expe = pers.tile([T, 1], F32, tag="expe")
            nc.scalar.activation(expe[:], endv[:], AF.Exp)

            tmpA = work.tile([T, 8], F32, tag="tmpA")
            nc.vector.tensor_scalar(
                tmpA[:], ebuf[:, 0:8], scalar1=exps[:, 0:1], scalar2=None,
                op0=OP.mult,
            )
            tmpB = work.tile([T, 8], F32, tag="tmpB")
            nc.vector.tensor_scalar(
                tmpB[:], ebuf[:, (S - 1) * 8:S * 8], scalar1=expe[:, 0:1],
                scalar2=None, op0=OP.mult,
            )
            TMID = S // 2 - 1  # 127
            psA = psB = None
            for i in range(TMID):  # A: t = 1..127 ; B: t = 254..128
                tA = 1 + i
                tB = S - 2 - i
                psA = ps_f.tile([T, 8], F32, tag="stf")
                nc.tensor.matmul(out=psA[:], lhsT=expT[:], rhs=tmpA[:],
                                 start=True, stop=True)
                psB = ps_b.tile([T, 8], F32, tag="stb")
                nc.tensor.matmul(out=psB[:], lhsT=expTT[:], rhs=tmpB[:],
                                 start=True, stop=True)
                tmpA = work.tile([T, 8], F32, tag="tmpA")
                nc.vector.tensor_tensor(
                    tmpA[:], psA[:], ebuf[:, tA * 8:(tA + 1) * 8], op=OP.mult
                )
                tmpB = work.tile([T, 8], F32, tag="tmpB")
                nc.vector.tensor_tensor(
                    tmpB[:], psB[:], ebuf[:, tB * 8:(tB + 1) * 8], op=OP.mult
                )
            # final B matmul: B_127 = expTT @ (E_128 * B_128)
            psB = ps_b.tile([T, 8], F32, tag="stb")
            nc.tensor.matmul(out=psB[:], lhsT=expTT[:], rhs=tmpB[:],
                             start=True, stop=True)
            ab = work.tile([T, 8], F32, tag="ab")
            nc.vector.tensor_tensor(ab[:], tmpA[:], psB[:], op=OP.mult)
            psZ = ps_tp.tile([1, 8], F32, tag="tp")
            nc.tensor.matmul(out=psZ[:], lhsT=ones9[:], rhs=ab[:],
                             start=True, stop=True)
            lz = pers.tile([1, 8], F32, tag="lz")
            nc.scalar.activation(lz[:], psZ[:], AF.Ln)
            diff = pers.tile([1, 8], F32, tag="diff")
            nc.vector.tensor_tensor(diff[:], lz[:], score_sb[:], op=OP.subtract)
            red = pers.tile([1, 1], F32, tag="red")
            nc.vector.tensor_reduce(red[:], diff[:], axis=mybir.AxisListType.X,
                                    op=OP.add)
            outc = pers.tile([1, 1], F32, tag="outc")
            nc.vector.tensor_scalar_add(outc[:], red[:], float(BL * S * MU))
            nc.sync.dma_start(out_d[:], outc[:])

    nc.finalize()
    return nc


def _prep_inputs(x, tags, crf_mask, embedding, W_ih_f, W_hh_f, b_f, W_ih_b,
                 W_hh_b, b_b, W_tag, b_tag, transitions, start_trans, end_trans):
    """Host-side sharding + layout prep. Pure reformatting / dtype casts."""
    x = np.asarray(x).astype(np.int32)
    tags = np.asarray(tags).astype(np.int32)
    mask = np.asarray(crf_mask)
    assert mask.all(), "kernel specialized to all-ones crf_mask"
    embedding = np.ascontiguousarray(np.asarray(embedding, dtype=np.float32))

    def perm_cols(w):  # [*, 4HD] -> gate-chunk permuted columns
        wc = w.reshape(w.shape[0], 8, 128)
        return np.ascontiguousarray(wc[:, PERM, :].reshape(w.shape[0], 4 * HD))

    wih = {"f": perm_cols(np.asarray(W_ih_f, np.float32).T),
           "b": perm_cols(np.asarray(W_ih_b, np.float32).T)}
    whh = {"f": perm_cols(np.asarray(W_hh_f, np.float32).T).astype(ml_dtypes.bfloat16),
           "b": perm_cols(np.asarray(W_hh_b, np.float32).T).astype(ml_dtypes.bfloat16)}
    brs = {}
    for d, b_ in (("f", b_f), ("b", b_b)):
        bv = np.asarray(b_, np.float32).reshape(8, 128)[PERM, :]  # [8,128]
        brs[d] = np.ascontiguousarray(bv.T)  # [128, 8]
    wtagT = np.ascontiguousarray(np.asarray(W_tag, np.float32).T).astype(
        ml_dtypes.bfloat16)  # [512, 9]
    btag = np.asarray(b_tag, np.float32).reshape(T, 1)
    startv = np.asarray(start_trans, np.float32).reshape(T, 1)
    endv = np.asarray(end_trans, np.float32).reshape(T, 1)
    transm = np.ascontiguousarray(np.asarray(transitions, np.float32))
    transmT = np.ascontiguousarray(transm.T)
    idf32 = np.eye(128, dtype=np.float32)
    idf16 = np.eye(128, dtype=np.float16)

    shared = {
        "emb": embedding, "wih_f": wih["f"], "wih_b": wih["b"],
        "whh_f": whh["f"], "whh_b": whh["b"], "br_f": brs["f"],
        "br_b": brs["b"], "wtagT": wtagT, "btag": btag, "startv": startv,
        "endv": endv, "transm": transm, "transmT": transmT,
        "idf32": idf32, "idf16": idf16,
    }

    in_maps = []
    tt = np.arange(TOK) // BL   # token -> t
    bb = np.arange(TOK) % BL    # token -> local b
    for c in range(NCORES):
        xc = x[c * BL:(c + 1) * BL]          # [8, 256]
        tc_ = tags[c * BL:(c + 1) * BL]      # [8, 256]
        idx = xc[bb, tt].astype(np.int32)    # [2048] token-major (t,b)
        idx_h = np.ascontiguousarray(idx.reshape(NCH, 128).T)  # [128, NCH]
        tag_tok = tc_[bb, tt]                # [2048]
        ohc = (tag_tok[None, :] == np.arange(T)[:, None]).astype(np.float32)
        nxt = np.full(TOK, -1, np.int64)
        nxt[: TOK - BL] = tag_tok[BL:]       # tag at (t+1, b); t=S-1 -> -1
        ohn = (nxt[None, :] == np.arange(T)[:, None]).astype(np.float32)
        m = dict(shared)
        m["idx"] = idx_h
        m["ohc"] = np.ascontiguousarray(ohc)
        m["ohn"] = np.ascontiguousarray(ohn)
        in_maps.append(m)
    return in_maps


def _run(inputs, trace=False):
    nc = _build(S)
    in_maps = _prep_inputs(**inputs)
    res = run_bass_kernel_spmd(
        nc, in_maps, core_ids=list(range(NCORES)), trace=trace
    )
    total = np.float64(0.0)
    for c in range(NCORES):
        total += np.float64(res.results[c]["out"][0, 0])
    return np.float32(total), res


def kernel(**inputs) -> np.ndarray:
    out, _ = _run(inputs, trace=False)
    return out
